# revision 8
# baseline (speedup 1.0000x reference)
"""Bass/Trainium2 kernel for nn_BlockGNN (2-layer GATv2 + MLP) on 8 NeuronCores.

Identity-scatter design: degree-sorted 128-dst windows with slot-position ==
dst-position in every edge tile, so the per-edge scatter/broadcast matrices
are constant identities (no sel/selT DMA). Feature-major GAT numerator comes
straight out of the scatter matmuls (no final transpose); softmax denominator
is a masked DVE reduce. fp16 datapath throughout; fp16 MLP.
"""

import os
import sys

import numpy as np

os.environ.setdefault("MYCRO_LOCAL_CACHE", "1")

for _p in ("/opt/trn_rl_repo",):
    if os.path.isdir(_p) and _p not in sys.path:
        sys.path.append(_p)

import concourse.bass as bass
import concourse.bacc as bacc
import concourse.mybir as mybir
import concourse.tile as tile
from concourse.bass import AP
from concourse.bass_utils import run_bass_kernel_spmd

F32 = mybir.dt.float32
F16 = mybir.dt.float16
FP8 = mybir.dt.float8e4

NPF32 = np.float32
NPF16 = np.float16
NPFP8 = mybir.dt.np(FP8)

N, E, D, H, CDIM, L = 50000, 800000, 128, 4, 32, 2
P = 128
NCORES = 8
NEG = 0.2
DEN_EPS = 1e-4

TRACE = bool(int(os.environ.get("KTRACE", "0")))

LAST_EXEC_NS = []
LAST_RESULTS = []


def _install_ntff_hook():
    try:
        import antenv.axon_hooks  # noqa: F401
        return
    except ImportError:
        pass
    import contextlib
    import ctypes
    import types

    try:
        import antenv
    except ImportError:
        return
    so_path = "/opt/axon/libaxon_pjrt.so"
    if not os.path.exists(so_path):
        return
    lib = ctypes.CDLL(so_path)
    if not hasattr(lib, "axon_start_nrt_profile"):
        return
    lib.axon_start_nrt_profile.argtypes = [
        ctypes.POINTER(ctypes.c_int64),
        ctypes.c_size_t,
    ]
    lib.axon_start_nrt_profile.restype = ctypes.c_int64
    lib.axon_stop_nrt_profile.argtypes = [ctypes.c_char_p]
    lib.axon_stop_nrt_profile.restype = ctypes.c_int64

    @contextlib.contextmanager
    def _hook(output_dir, device_ids):
        import jax

        jax.devices()
        if device_ids:
            ids = (ctypes.c_int64 * len(device_ids))(*device_ids)
            rc = lib.axon_start_nrt_profile(ids, len(device_ids))
        else:
            rc = lib.axon_start_nrt_profile(None, 0)
        if rc != 0:
            raise RuntimeError(f"axon_start_nrt_profile rc={rc}")
        try:
            yield
        finally:
            n = lib.axon_stop_nrt_profile(str(output_dir).encode())
            print(f"ntff profile: {n} file(s) -> {output_dir}", file=sys.stderr)

    mod = types.ModuleType("antenv.axon_hooks")
    _state = {"hook": _hook}
    mod.get_axon_ntff_profile_hook = lambda: _state["hook"]
    mod.set_axon_ntff_profile_hook = lambda h: _state.update(hook=h)
    sys.modules["antenv.axon_hooks"] = mod
    antenv.axon_hooks = mod


if TRACE:
    _install_ntff_hook()


def _bcast_last(ap: AP, n: int) -> AP:
    return AP(ap.tensor, ap.offset, [list(p) for p in ap.ap] + [[0, n]])


def _macros(kt):
    out = []
    j0 = 0
    while j0 < kt:
        wdt = 4 if kt - j0 >= 4 else kt - j0
        out.append((j0, wdt))
        j0 += wdt
    return out


def build_layer_nc(cfg):
    KTPROF = cfg["ktprof"]  # list of per-window tile counts (len NWIN)
    NWIN = len(KTPROF)
    KTMAX = max(KTPROF)
    KTSUM = sum(KTPROF)
    ESLOT = KTSUM * P
    NLOCP = NWIN * P

    nc = bacc.Bacc(
        "TRN2",
        target_bir_lowering=False,
        debug=False,
        enable_asserts=False,
        num_devices=cfg.get("ncores", NCORES),
    )

    xgT = nc.dram_tensor("xgT", [P, ESLOT], F16, kind="ExternalInput").ap()
    eaT = nc.dram_tensor("eaT", [CDIM, ESLOT], F16, kind="ExternalInput").ap()
    maskd = nc.dram_tensor("maskd", [P, 4 * KTSUM], F16, kind="ExternalInput").ap()
    xTloc = nc.dram_tensor("xTloc", [P, NLOCP], F16, kind="ExternalInput").ap()
    Wl_b = nc.dram_tensor("Wl_b", [P, P], F16, kind="ExternalInput").ap()
    Wr = nc.dram_tensor("Wr", [P, P], F16, kind="ExternalInput").ap()
    We_b = nc.dram_tensor("We_b", [CDIM, P], F16, kind="ExternalInput").ap()
    w1 = nc.dram_tensor("w1", [P, P], F16, kind="ExternalInput").ap()
    w2 = nc.dram_tensor("w2", [P, P], F16, kind="ExternalInput").ap()
    attcol = nc.dram_tensor("attcol", [P, 1], F32, kind="ExternalInput").ap()
    pbias = nc.dram_tensor("pbias", [P, 1], F32, kind="ExternalInput").ap()
    sgn4 = nc.dram_tensor("sgn4", [P, 4], F16, kind="ExternalInput").ap()
    ident = nc.dram_tensor("ident", [P, 4 * P], FP8, kind="ExternalInput").ap()
    hmask = nc.dram_tensor("hmask", [4, P], FP8, kind="ExternalInput").ap()
    b1c = nc.dram_tensor("b1c", [P, 1], F32, kind="ExternalInput").ap()
    b2c = nc.dram_tensor("b2c", [P, 1], F32, kind="ExternalInput").ap()
    xoutT = nc.dram_tensor("xoutT", [P, NLOCP], F32, kind="ExternalOutput").ap()

    AF = mybir.ActivationFunctionType
    OP = mybir.AluOpType
    AX = mybir.AxisListType

    with tile.TileContext(nc) as tc:
        with (
            tc.tile_pool(name="const", bufs=1) as cpool,
            tc.tile_pool(name="win", bufs=3) as wpool,
            tc.tile_pool(name="edge", bufs=3) as epool,
            tc.tile_pool(name="psZ", bufs=2, space="PSUM") as psZ,
            tc.tile_pool(name="psV", bufs=2, space="PSUM") as psV,
            tc.tile_pool(name="psM", bufs=1, space="PSUM") as psM,
            tc.tile_pool(name="psO", bufs=2, space="PSUM") as psO,
            tc.tile_pool(name="psE", bufs=1, space="PSUM") as psE,
        ):
            def cload(ap, shape, dt, tag):
                t = cpool.tile(shape, dt, tag=tag)
                nc.sync.dma_start(out=t[:], in_=ap)
                return t

            Wlb_s = cload(Wl_b, [P, P], F16, tag="Wlb_s")
            Wr_s = cload(Wr, [P, P], F16, tag="Wr_s")
            We_s = cload(We_b, [CDIM, P], F16, tag="We_s")
            w1_s = cload(w1, [P, P], F16, tag="w1_s")
            w2_s = cload(w2, [P, P], F16, tag="w2_s")
            attc_s = cload(attcol, [P, 1], F32, tag="attc_s")
            pbias_s = cload(pbias, [P, 1], F32, tag="pbias_s")
            sgn_s = cload(sgn4, [P, 4], F16, tag="sgn_s")
            ident_s = cload(ident, [P, 4 * P], FP8, tag="ident_s")
            hmask_s = cload(hmask, [4, P], FP8, tag="hmask_s")
            b1c_s = cload(b1c, [P, 1], F32, tag="b1c_s")
            b2c_s = cload(b2c, [P, 1], F32, tag="b2c_s")

            pending_epilogue = [None]

            def emit_epilogue():
                fn = pending_epilogue[0]
                if fn is not None:
                    pending_epilogue[0] = None
                    fn()

            ktoff = [0]
            for k in KTPROF:
                ktoff.append(ktoff[-1] + k)

            worder = sorted(range(NWIN), key=lambda i: KTPROF[i])
            for w in worder:
                KT = KTPROF[w]
                S = KT * P
                MACROS = _macros(KT)
                NMAC = len(MACROS)
                so = ktoff[w] * P  # slot offset
                mo = ktoff[w] * 4  # mask/exp offset

                xgT_sb = wpool.tile([P, KTMAX * P], F16, tag="xgT")
                nc.sync.dma_start(out=xgT_sb[:, :S], in_=xgT[:, so : so + S])
                eaT_sb = wpool.tile([CDIM, KTMAX * P], F16, tag="ea")
                nc.sync.dma_start(out=eaT_sb[:, :S], in_=eaT[:, so : so + S])
                mask_sb = wpool.tile([P, KTMAX * 4], F16, tag="mask")
                nc.sync.dma_start(
                    out=mask_sb[:, : KT * 4], in_=maskd[:, mo : mo + KT * 4]
                )
                xtl_sb = wpool.tile([P, P], F16, tag="xtl")
                nc.sync.dma_start(out=xtl_sb[:], in_=xTloc[:, w * P : (w + 1) * P])

                # xr edge(dst)-major: [d, f] = (x_win^T @ Wr)
                xr_ps = psE.tile([P, P], F32, tag="epi")
                nc.tensor.matmul(out=xr_ps[:], lhsT=xtl_sb[:], rhs=Wr_s[:],
                                 start=True, stop=True)
                xr_sb = wpool.tile([P, P], F16, tag="xrs")
                nc.vector.tensor_copy(xr_sb[:], xr_ps[:])

                exs_sb = wpool.tile([P, KTMAX * 4], F16, tag="exs")
                numFM = psO.tile([P, P], F32, tag="num")

                state = [None] * NMAC

                def stage_alpha(mi):
                    j0, MW, vq, am, comb = state[mi]
                    mini = psM.tile([P, 16], F32, tag="mini")
                    for u in range(MW):
                        nc.tensor.matmul(
                            out=mini[:, u * 4 : (u + 1) * 4],
                            lhsT=am[:, u * P : (u + 1) * P],
                            rhs=sgn_s[:],
                            start=(u == 0),
                            stop=(u == MW - 1),
                        )
                    exv = exs_sb[:, j0 * 4 : (j0 + MW) * 4]
                    nc.scalar.activation(exv, mini[:, : MW * 4], AF.Exp)
                    nc.vector.tensor_tensor(
                        comb[:, : MW * P].rearrange("p (b h c) -> p b h c",
                                                    b=MW, c=CDIM),
                        vq[:, : MW * P].rearrange("p (b h c) -> p b h c",
                                                  b=MW, c=CDIM),
                        _bcast_last(exv.rearrange("p (b h) -> p b h", h=4), CDIM),
                        op=OP.mult,
                    )

                def stage_scatter(mi):
                    j0, MW, vq, am, comb = state[mi]
                    for u in range(MW):
                        nc.tensor.matmul(
                            out=numFM[:],
                            lhsT=comb[:, u * P : (u + 1) * P],
                            rhs=ident_s[:, :P],
                            start=(mi == 0 and u == 0),
                            stop=(mi == NMAC - 1 and u == MW - 1),
                        )

                for mi, (j0, MW) in enumerate(MACROS):
                    SM = MW * P
                    zq = psZ.tile([P, 4 * P], F32, tag="zq")
                    nc.tensor.matmul(
                        out=zq[:, :SM],
                        lhsT=Wlb_s[:],
                        rhs=xgT_sb[:, j0 * P : j0 * P + SM],
                        start=True, stop=False,
                    )
                    nc.tensor.matmul(
                        out=zq[:, :SM],
                        lhsT=We_s[:],
                        rhs=eaT_sb[:, j0 * P : j0 * P + SM],
                        start=False, stop=False,
                    )
                    nc.tensor.matmul(
                        out=zq[:, :SM],
                        lhsT=xr_sb[:],
                        rhs=ident_s[:, :SM],
                        start=False, stop=True,
                    )
                    am = epool.tile([P, 4 * P], F16, tag="am")
                    nc.scalar.activation(am[:, :SM], zq[:, :SM], AF.Prelu,
                                         scale=attc_s[:], bias=pbias_s[:],
                                         alpha=NEG)
                    vq = psV.tile([P, 4 * P], F32, tag="vq")
                    for u in range(MW):
                        j = j0 + u
                        nc.tensor.matmul(
                            out=vq[:, u * P : (u + 1) * P],
                            lhsT=xgT_sb[:, j * P : (j + 1) * P],
                            rhs=Wlb_s[:],
                            start=(u == 0),
                            stop=(u == MW - 1),
                        )
                    comb = epool.tile([P, 4 * P], F16, tag="comb")
                    state[mi] = (j0, MW, vq, am, comb)

                    if mi == 0:
                        emit_epilogue()
                    if mi >= 1:
                        stage_alpha(mi - 1)
                    if mi >= 2:
                        stage_scatter(mi - 2)

                stage_alpha(NMAC - 1)
                for k in range(max(0, NMAC - 2), NMAC):
                    stage_scatter(k)

                def make_epilogue(w, KT, numFM, exs_sb, mask_sb):
                    def epi():
                        # masked softmax denominator per (dst, head)
                        exm = wpool.tile([P, KTMAX * 4], F16, tag="exm")
                        nc.vector.tensor_tensor(
                            exm[:, : KT * 4], exs_sb[:, : KT * 4],
                            mask_sb[:, : KT * 4], op=OP.mult,
                        )
                        den = wpool.tile([P, 4], F32, tag="den")
                        nc.vector.tensor_reduce(
                            den[:],
                            exm[:, : KT * 4].rearrange("p (b h) -> p h b", h=4),
                            AX.X,
                            OP.add,
                        )
                        de = wpool.tile([P, 4], F32, tag="de")
                        nc.vector.tensor_scalar(de[:], den[:], DEN_EPS, None,
                                                OP.add)
                        rc = wpool.tile([P, 4], F16, tag="rc")
                        with nc.allow_low_precision(reason="fp16 rc validated"):
                            nc.vector.reciprocal(rc[:], de[:])
                        # rcm[f, d] = rc[d, head(f)] via two tiny matmuls
                        rcT_ps = psE.tile([P, P], F32, tag="epi")
                        nc.tensor.matmul(out=rcT_ps[:4, :], lhsT=rc[:],
                                         rhs=ident_s[:, :P],
                                         start=True, stop=True)
                        rcT_sb = wpool.tile([4, P], F16, tag="rcT")
                        nc.vector.tensor_copy(rcT_sb[:], rcT_ps[:4, :])
                        rcm_ps = psE.tile([P, P], F32, tag="epi")
                        nc.tensor.matmul(out=rcm_ps[:], lhsT=hmask_s[:],
                                         rhs=rcT_sb[:], start=True, stop=True)
                        rcm_sb = wpool.tile([P, P], F16, tag="rcm")
                        nc.vector.tensor_copy(rcm_sb[:], rcm_ps[:])
                        gTb = wpool.tile([P, P], F16, tag="gTb")
                        nc.vector.tensor_tensor(gTb[:], numFM[:], rcm_sb[:],
                                                op=OP.mult)
                        y1_ps = psE.tile([P, P], F32, tag="epi")
                        nc.tensor.matmul(out=y1_ps[:], lhsT=w1_s[:], rhs=gTb[:],
                                         start=True, stop=True)
                        y1s = wpool.tile([P, P], F16, tag="y1s")
                        nc.scalar.activation(y1s[:], y1_ps[:], AF.Relu,
                                             bias=b1c_s[:])
                        y2_ps = psE.tile([P, P], F32, tag="epi")
                        nc.tensor.matmul(out=y2_ps[:], lhsT=w2_s[:], rhs=y1s[:],
                                         start=True, stop=True)
                        xo = wpool.tile([P, P], F32, tag="xo")
                        nc.vector.tensor_scalar(xo[:], y2_ps[:], b2c_s[:],
                                                None, OP.add)
                        nc.sync.dma_start(out=xoutT[:, w * P : (w + 1) * P],
                                          in_=xo[:])
                    return epi

                pending_epilogue[0] = make_epilogue(w, KT, numFM, exs_sb, mask_sb)

            emit_epilogue()

    nc.compile()
    return nc


def _preprocess(edge_index, edge_attr, ncores, nloc, nwin):
    """Degree-sorted identity-slot layout.

    Each core owns nloc dst nodes (snake-dealt by degree so cores balance).
    Within a core, dsts are sorted by degree desc; window w takes dsts
    [w*128, (w+1)*128), column r = rank within window. Edge j of the dst in
    column r sits at slot offs[w] + j*128 + r, so slot position == dst
    position in every 128-slot tile (scatter matrices become identity).
    KT profile is the per-window max tile count across cores (SPMD).
    """
    src = np.ascontiguousarray(edge_index[0]).astype(np.int64)
    dst = np.ascontiguousarray(edge_index[1]).astype(np.int64)
    n = nloc * ncores
    ea = np.ascontiguousarray(edge_attr, dtype=np.float32)

    deg = np.bincount(dst, minlength=n).astype(np.int64)
    deg2 = deg + 1  # + self loop

    # snake-deal ownership by degree
    dorder = np.argsort(-deg2, kind="stable")
    ranks = np.arange(n)
    blk = ranks // ncores
    cseq = np.where(blk % 2 == 0, ranks % ncores,
                    ncores - 1 - (ranks % ncores))
    owner = np.empty(n, np.int64)
    owner[dorder] = cseq

    # self-loop attr = mean of incoming edge attrs
    order = np.argsort(dst, kind="stable")
    dst_s = dst[order]
    src_s = src[order]
    ea_s = ea[order]
    cs = np.concatenate(
        [np.zeros((1, ea.shape[1]), np.float64),
         np.cumsum(ea_s, 0, dtype=np.float64)]
    )
    starts = np.searchsorted(dst_s, np.arange(n))
    ends = np.searchsorted(dst_s, np.arange(n) + 1)
    loop_attr = ((cs[ends] - cs[starts])
                 / np.maximum(deg, 1).astype(np.float64)[:, None]).astype(
        np.float32
    )

    # per-core window layout
    percore = []
    profs = np.zeros((ncores, nwin), np.int64)
    for c in range(ncores):
        own = np.where(owner == c)[0]
        o = np.argsort(-deg2[own], kind="stable")
        perm = own[o]  # global node id per column (window-major)
        d2 = deg2[perm]
        for w in range(nwin):
            seg = d2[w * P : (w + 1) * P]
            profs[c, w] = seg[0] if len(seg) else 1
        percore.append((perm, d2))

    ktprof = profs.max(0)
    ktprof = np.maximum(ktprof, 1)
    ktoff = np.concatenate([[0], np.cumsum(ktprof)])
    eslot = int(ktoff[-1]) * P

    data = []
    for c in range(ncores):
        perm, d2 = percore[c]
        ncol = len(perm)
        src_slot = np.zeros(eslot, np.int64)
        valid = np.zeros(eslot, bool)
        ea_slot = np.zeros((eslot, CDIM), np.float32)

        col_w = np.arange(ncol) // P
        col_r = np.arange(ncol) % P
        # edges grouped by dst: dst_s[starts[g]:ends[g]] are node g's edges
        for i in range(ncol):
            g = perm[i]
            w, r = col_w[i], col_r[i]
            base = int(ktoff[w]) * P + r
            k = int(deg[g])
            sl = base + np.arange(k) * P
            src_slot[sl] = src_s[starts[g] : ends[g]]
            ea_slot[sl] = ea_s[starts[g] : ends[g]]
            # self loop at edge index k
            sl2 = base + k * P
            src_slot[sl2] = g
            ea_slot[sl2] = loop_attr[g]
            valid[sl] = True
            valid[sl2] = True
        m = np.zeros((P, int(ktprof.sum()), 4), np.float16)
        for i in range(ncol):
            w, r = col_w[i], col_r[i]
            m[r, int(ktoff[w]) : int(ktoff[w]) + int(d2[i]), :] = 1.0
        mask01 = m.reshape(P, -1)

        eaT = np.ascontiguousarray(ea_slot.T).astype(NPF16)
        data.append(dict(src_slot=src_slot, valid=valid, eaT=eaT,
                         mask01=mask01, perm=perm))
    return data, [int(k) for k in ktprof]


def _layer_weight_maps(inputs, layer):
    i = layer
    att = np.asarray(inputs["att"])
    attf = att[i].reshape(-1).astype(np.float32)
    sgn = np.zeros((P, H), np.float32)
    for h in range(H):
        sgn[h * CDIM : (h + 1) * CDIM, h] = np.sign(
            attf[h * CDIM : (h + 1) * CDIM]
        )
    idt = np.zeros((P, 4 * P), NPFP8)
    eye = np.eye(P, dtype=NPFP8)
    for k in range(4):
        idt[:, k * P : (k + 1) * P] = eye
    hm = np.zeros((4, P), NPFP8)
    for h in range(H):
        hm[h, h * CDIM : (h + 1) * CDIM] = 1.0
    bgc = (np.asarray(inputs["bias"][i]) + np.asarray(inputs["bl"][i])).astype(
        np.float32
    )
    w1f = np.asarray(inputs["w1"][i]).astype(np.float32)
    b1p = np.asarray(inputs["b1"][i]).astype(np.float32) + bgc @ w1f
    m = dict(
        Wl_b=np.ascontiguousarray(inputs["Wl"][i]).astype(NPF16),
        Wr=np.ascontiguousarray(inputs["Wr"][i]).astype(NPF16),
        We_b=np.ascontiguousarray(inputs["We"][i]).astype(NPF16),
        w1=w1f.astype(NPF16),
        w2=np.ascontiguousarray(inputs["w2"][i]).astype(NPF16),
        attcol=np.abs(attf).reshape(P, 1).astype(NPF32),
        pbias=(np.abs(attf)
               * (np.asarray(inputs["br"][i]) + np.asarray(inputs["bl"][i])))
        .reshape(P, 1)
        .astype(NPF32),
        sgn4=sgn.astype(NPF16),
        ident=idt,
        hmask=hm,
        b1c=b1p.reshape(P, 1).astype(NPF32),
        b2c=np.asarray(inputs["b2"][i]).reshape(P, 1).astype(NPF32),
    )
    return m


_NC_CACHE = {}


def kernel(**inputs):
    nodes = np.asarray(inputs["nodes"], dtype=np.float32)
    edge_index = np.asarray(inputs["edge_index"])
    edge_attr = np.asarray(inputs["edge_attr"], dtype=np.float32)

    n, d = nodes.shape
    assert (n, d) == (N, D)
    nloc = n // NCORES
    nwin = -(-nloc // P)

    data, ktprof = _preprocess(edge_index, edge_attr, NCORES, nloc, nwin)

    key = (tuple(ktprof), NCORES)
    if key not in _NC_CACHE:
        _NC_CACHE[key] = build_layer_nc(dict(ktprof=ktprof, ncores=NCORES))
    nc = _NC_CACHE[key]

    x_curr = np.ascontiguousarray(nodes.T)  # [D, N] f32

    nlocp = nwin * P
    for layer in range(L):
        wmap = _layer_weight_maps(inputs, layer)
        xce = x_curr.astype(NPF16)
        in_maps = []
        for c in range(NCORES):
            dd = data[c]
            perm = dd["perm"]
            xTloc = np.zeros((P, nlocp), NPF16)
            xTloc[:, : len(perm)] = xce[:, perm]
            xgT = xce[:, dd["src_slot"]]
            xgT[:, ~dd["valid"]] = 0
            m = dict(wmap)
            m["xgT"] = np.ascontiguousarray(xgT)
            m["xTloc"] = xTloc
            m["eaT"] = dd["eaT"]
            m["maskd"] = dd["mask01"]
            in_maps.append(m)
        res = run_bass_kernel_spmd(
            nc, in_maps, core_ids=list(range(NCORES)), trace=TRACE
        )
        if res.exec_time_ns is not None:
            LAST_EXEC_NS.append(res.exec_time_ns)
        if TRACE:
            LAST_RESULTS.append(res)
        outs = res.results
        x_next = np.zeros((P, n), NPF32)
        for c in range(NCORES):
            perm = data[c]["perm"]
            x_next[:, perm] = outs[c]["xoutT"][:, : len(perm)]
        x_curr = x_next

    return np.ascontiguousarray(x_curr.T.astype(np.float32))


# revision 9
# speedup vs baseline: 1.1895x; 1.1895x over previous
"""Bass/Trainium2 kernel for nn_BlockGNN (2-layer GATv2 + MLP) on 8 NeuronCores.

Identity-scatter design: degree-sorted 128-dst windows with slot-position ==
dst-position in every edge tile, so the per-edge scatter/broadcast matrices
are constant identities (no sel/selT DMA). Feature-major GAT numerator comes
straight out of the scatter matmuls (no final transpose); softmax denominator
is a masked DVE reduce. fp16 datapath throughout; fp16 MLP.
"""

import os
import sys

import numpy as np

os.environ.setdefault("MYCRO_LOCAL_CACHE", "1")

for _p in ("/opt/trn_rl_repo",):
    if os.path.isdir(_p) and _p not in sys.path:
        sys.path.append(_p)

import concourse.bass as bass
import concourse.bacc as bacc
import concourse.mybir as mybir
import concourse.tile as tile
from concourse.bass import AP
from concourse.bass_utils import run_bass_kernel_spmd

F32 = mybir.dt.float32
F16 = mybir.dt.float16
FP8 = mybir.dt.float8e4

NPF32 = np.float32
NPF16 = np.float16
NPFP8 = mybir.dt.np(FP8)

N, E, D, H, CDIM, L = 50000, 800000, 128, 4, 32, 2
P = 128
NCORES = 8
NEG = 0.2
DEN_EPS = 1e-4

TRACE = bool(int(os.environ.get("KTRACE", "0")))

LAST_EXEC_NS = []
LAST_RESULTS = []


def _install_ntff_hook():
    try:
        import antenv.axon_hooks  # noqa: F401
        return
    except ImportError:
        pass
    import contextlib
    import ctypes
    import types

    try:
        import antenv
    except ImportError:
        return
    so_path = "/opt/axon/libaxon_pjrt.so"
    if not os.path.exists(so_path):
        return
    lib = ctypes.CDLL(so_path)
    if not hasattr(lib, "axon_start_nrt_profile"):
        return
    lib.axon_start_nrt_profile.argtypes = [
        ctypes.POINTER(ctypes.c_int64),
        ctypes.c_size_t,
    ]
    lib.axon_start_nrt_profile.restype = ctypes.c_int64
    lib.axon_stop_nrt_profile.argtypes = [ctypes.c_char_p]
    lib.axon_stop_nrt_profile.restype = ctypes.c_int64

    @contextlib.contextmanager
    def _hook(output_dir, device_ids):
        import jax

        jax.devices()
        if device_ids:
            ids = (ctypes.c_int64 * len(device_ids))(*device_ids)
            rc = lib.axon_start_nrt_profile(ids, len(device_ids))
        else:
            rc = lib.axon_start_nrt_profile(None, 0)
        if rc != 0:
            raise RuntimeError(f"axon_start_nrt_profile rc={rc}")
        try:
            yield
        finally:
            n = lib.axon_stop_nrt_profile(str(output_dir).encode())
            print(f"ntff profile: {n} file(s) -> {output_dir}", file=sys.stderr)

    mod = types.ModuleType("antenv.axon_hooks")
    _state = {"hook": _hook}
    mod.get_axon_ntff_profile_hook = lambda: _state["hook"]
    mod.set_axon_ntff_profile_hook = lambda h: _state.update(hook=h)
    sys.modules["antenv.axon_hooks"] = mod
    antenv.axon_hooks = mod


if TRACE:
    _install_ntff_hook()


def _bcast_last(ap: AP, n: int) -> AP:
    return AP(ap.tensor, ap.offset, [list(p) for p in ap.ap] + [[0, n]])


def _macros(kt):
    out = []
    j0 = 0
    while j0 < kt:
        wdt = 4 if kt - j0 >= 4 else kt - j0
        out.append((j0, wdt))
        j0 += wdt
    return out


def build_layer_nc(cfg):
    KTPROF = cfg["ktprof"]  # list of per-window tile counts (len NWIN)
    NWIN = len(KTPROF)
    KTMAX = max(KTPROF)
    KTSUM = sum(KTPROF)
    ESLOT = KTSUM * P
    NLOCP = NWIN * P

    nc = bacc.Bacc(
        "TRN2",
        target_bir_lowering=False,
        debug=False,
        enable_asserts=False,
        num_devices=cfg.get("ncores", NCORES),
    )

    xgT = nc.dram_tensor("xgT", [P, ESLOT], F16, kind="ExternalInput").ap()
    eaT = nc.dram_tensor("eaT", [CDIM, ESLOT], F16, kind="ExternalInput").ap()
    maskd = nc.dram_tensor("maskd", [P, 4 * KTSUM], F16, kind="ExternalInput").ap()
    xTloc = nc.dram_tensor("xTloc", [P, NLOCP], F16, kind="ExternalInput").ap()
    Wl_b = nc.dram_tensor("Wl_b", [P, P], F16, kind="ExternalInput").ap()
    Wr = nc.dram_tensor("Wr", [P, P], F16, kind="ExternalInput").ap()
    We_b = nc.dram_tensor("We_b", [CDIM, P], F16, kind="ExternalInput").ap()
    w1 = nc.dram_tensor("w1", [P, P], F16, kind="ExternalInput").ap()
    w2 = nc.dram_tensor("w2", [P, P], F16, kind="ExternalInput").ap()
    attcol = nc.dram_tensor("attcol", [P, 1], F32, kind="ExternalInput").ap()
    pbias = nc.dram_tensor("pbias", [P, 1], F32, kind="ExternalInput").ap()
    sgn4 = nc.dram_tensor("sgn4", [P, 4], F16, kind="ExternalInput").ap()
    ident = nc.dram_tensor("ident", [P, 4 * P], FP8, kind="ExternalInput").ap()
    hmask = nc.dram_tensor("hmask", [4, P], FP8, kind="ExternalInput").ap()
    b1c = nc.dram_tensor("b1c", [P, 1], F32, kind="ExternalInput").ap()
    b2c = nc.dram_tensor("b2c", [P, 1], F32, kind="ExternalInput").ap()
    xoutT = nc.dram_tensor("xoutT", [P, NLOCP], F32, kind="ExternalOutput").ap()

    AF = mybir.ActivationFunctionType
    OP = mybir.AluOpType
    AX = mybir.AxisListType

    with tile.TileContext(nc) as tc:
        with (
            tc.tile_pool(name="const", bufs=1) as cpool,
            tc.tile_pool(name="win", bufs=3) as wpool,
            tc.tile_pool(name="edge", bufs=3) as epool,
            tc.tile_pool(name="psZ", bufs=2, space="PSUM") as psZ,
            tc.tile_pool(name="psV", bufs=3, space="PSUM") as psV,
            tc.tile_pool(name="psM", bufs=1, space="PSUM") as psM,
            tc.tile_pool(name="psO", bufs=1, space="PSUM") as psO,
            tc.tile_pool(name="psE", bufs=1, space="PSUM") as psE,
        ):
            def cload(ap, shape, dt, tag):
                t = cpool.tile(shape, dt, tag=tag)
                nc.sync.dma_start(out=t[:], in_=ap)
                return t

            Wlb_s = cload(Wl_b, [P, P], F16, tag="Wlb_s")
            Wr_s = cload(Wr, [P, P], F16, tag="Wr_s")
            We_s = cload(We_b, [CDIM, P], F16, tag="We_s")
            w1_s = cload(w1, [P, P], F16, tag="w1_s")
            w2_s = cload(w2, [P, P], F16, tag="w2_s")
            attc_s = cload(attcol, [P, 1], F32, tag="attc_s")
            pbias_s = cload(pbias, [P, 1], F32, tag="pbias_s")
            sgn_s = cload(sgn4, [P, 4], F16, tag="sgn_s")
            ident_s = cload(ident, [P, 4 * P], FP8, tag="ident_s")
            hmask_s = cload(hmask, [4, P], FP8, tag="hmask_s")
            b1c_s = cload(b1c, [P, 1], F32, tag="b1c_s")
            b2c_s = cload(b2c, [P, 1], F32, tag="b2c_s")

            pending_epilogue = [None]

            def emit_epilogue():
                fn = pending_epilogue[0]
                if fn is not None:
                    pending_epilogue[0] = None
                    fn()

            ktoff = [0]
            for k in KTPROF:
                ktoff.append(ktoff[-1] + k)

            worder = sorted(range(NWIN), key=lambda i: KTPROF[i])
            for w in worder:
                KT = KTPROF[w]
                S = KT * P
                MACROS = _macros(KT)
                NMAC = len(MACROS)
                so = ktoff[w] * P  # slot offset
                mo = ktoff[w] * 4  # mask/exp offset

                xgT_sb = wpool.tile([P, KTMAX * P], F16, tag="xgT")
                nc.sync.dma_start(out=xgT_sb[:, :S], in_=xgT[:, so : so + S])
                eaT_sb = wpool.tile([CDIM, KTMAX * P], F16, tag="ea")
                nc.sync.dma_start(out=eaT_sb[:, :S], in_=eaT[:, so : so + S])
                mask_sb = wpool.tile([P, KTMAX * 4], F16, tag="mask")
                nc.sync.dma_start(
                    out=mask_sb[:, : KT * 4], in_=maskd[:, mo : mo + KT * 4]
                )
                xtl_sb = wpool.tile([P, P], F16, tag="xtl")
                nc.sync.dma_start(out=xtl_sb[:], in_=xTloc[:, w * P : (w + 1) * P])

                # xr edge(dst)-major: [d, f] = (x_win^T @ Wr)
                xr_ps = psE.tile([P, P], F32, tag="epi")
                nc.tensor.matmul(out=xr_ps[:], lhsT=xtl_sb[:], rhs=Wr_s[:],
                                 start=True, stop=True)
                xr_sb = wpool.tile([P, P], F16, tag="xrs")
                nc.vector.tensor_copy(xr_sb[:], xr_ps[:])

                exs_sb = wpool.tile([P, KTMAX * 4], F16, tag="exs")
                numFM = psO.tile([P, P], F32, tag="num")

                state = [None] * NMAC

                def stage_alpha(mi):
                    j0, MW, vq, am, comb = state[mi]
                    mini = psM.tile([P, 16], F32, tag="mini")
                    for u in range(MW):
                        nc.tensor.matmul(
                            out=mini[:, u * 4 : (u + 1) * 4],
                            lhsT=am[:, u * P : (u + 1) * P],
                            rhs=sgn_s[:],
                            start=(u == 0),
                            stop=(u == MW - 1),
                        )
                    exv = exs_sb[:, j0 * 4 : (j0 + MW) * 4]
                    nc.scalar.activation(exv, mini[:, : MW * 4], AF.Exp)
                    nc.vector.tensor_tensor(
                        comb[:, : MW * P].rearrange("p (b h c) -> p b h c",
                                                    b=MW, c=CDIM),
                        vq[:, : MW * P].rearrange("p (b h c) -> p b h c",
                                                  b=MW, c=CDIM),
                        _bcast_last(exv.rearrange("p (b h) -> p b h", h=4), CDIM),
                        op=OP.mult,
                    )

                def stage_scatter(mi):
                    j0, MW, vq, am, comb = state[mi]
                    for u in range(MW):
                        nc.tensor.matmul(
                            out=numFM[:],
                            lhsT=comb[:, u * P : (u + 1) * P],
                            rhs=ident_s[:, :P],
                            start=(mi == 0 and u == 0),
                            stop=(mi == NMAC - 1 and u == MW - 1),
                        )

                for mi, (j0, MW) in enumerate(MACROS):
                    SM = MW * P
                    zq = psZ.tile([P, 4 * P], F32, tag="zq")
                    nc.tensor.matmul(
                        out=zq[:, :SM],
                        lhsT=Wlb_s[:],
                        rhs=xgT_sb[:, j0 * P : j0 * P + SM],
                        start=True, stop=False,
                    )
                    nc.tensor.matmul(
                        out=zq[:, :SM],
                        lhsT=We_s[:],
                        rhs=eaT_sb[:, j0 * P : j0 * P + SM],
                        start=False, stop=False,
                    )
                    nc.tensor.matmul(
                        out=zq[:, :SM],
                        lhsT=xr_sb[:],
                        rhs=ident_s[:, :SM],
                        start=False, stop=True,
                    )
                    am = epool.tile([P, 4 * P], F16, tag="am")
                    nc.scalar.activation(am[:, :SM], zq[:, :SM], AF.Prelu,
                                         scale=attc_s[:], bias=pbias_s[:],
                                         alpha=NEG)
                    vq = psV.tile([P, 4 * P], F32, tag="vq")
                    for u in range(MW):
                        j = j0 + u
                        nc.tensor.matmul(
                            out=vq[:, u * P : (u + 1) * P],
                            lhsT=xgT_sb[:, j * P : (j + 1) * P],
                            rhs=Wlb_s[:],
                            start=(u == 0),
                            stop=(u == MW - 1),
                        )
                    comb = epool.tile([P, 4 * P], F16, tag="comb")
                    state[mi] = (j0, MW, vq, am, comb)

                    if mi == 0:
                        emit_epilogue()
                    if mi >= 1:
                        stage_alpha(mi - 1)
                    if mi >= 2:
                        stage_scatter(mi - 2)

                stage_alpha(NMAC - 1)
                for k in range(max(0, NMAC - 2), NMAC):
                    stage_scatter(k)

                def make_epilogue(w, KT, numFM, exs_sb, mask_sb):
                    def epi():
                        # masked softmax denominator per (dst, head)
                        exm = wpool.tile([P, KTMAX * 4], F16, tag="exm")
                        nc.vector.tensor_tensor(
                            exm[:, : KT * 4], exs_sb[:, : KT * 4],
                            mask_sb[:, : KT * 4], op=OP.mult,
                        )
                        den = wpool.tile([P, 4], F32, tag="den")
                        nc.vector.tensor_reduce(
                            den[:],
                            exm[:, : KT * 4].rearrange("p (b h) -> p h b", h=4),
                            AX.X,
                            OP.add,
                        )
                        de = wpool.tile([P, 4], F32, tag="de")
                        nc.vector.tensor_scalar(de[:], den[:], DEN_EPS, None,
                                                OP.add)
                        rc = wpool.tile([P, 4], F16, tag="rc")
                        with nc.allow_low_precision(reason="fp16 rc validated"):
                            nc.vector.reciprocal(rc[:], de[:])
                        # rcm[f, d] = rc[d, head(f)] via two tiny matmuls
                        rcT_ps = psE.tile([P, P], F32, tag="epi")
                        nc.tensor.matmul(out=rcT_ps[:4, :], lhsT=rc[:],
                                         rhs=ident_s[:, :P],
                                         start=True, stop=True)
                        rcT_sb = wpool.tile([4, P], F16, tag="rcT")
                        nc.vector.tensor_copy(rcT_sb[:], rcT_ps[:4, :])
                        rcm_ps = psE.tile([P, P], F32, tag="epi")
                        nc.tensor.matmul(out=rcm_ps[:], lhsT=hmask_s[:],
                                         rhs=rcT_sb[:], start=True, stop=True)
                        rcm_sb = wpool.tile([P, P], F16, tag="rcm")
                        nc.vector.tensor_copy(rcm_sb[:], rcm_ps[:])
                        gTb = wpool.tile([P, P], F16, tag="gTb")
                        nc.vector.tensor_tensor(gTb[:], numFM[:], rcm_sb[:],
                                                op=OP.mult)
                        y1_ps = psE.tile([P, P], F32, tag="epi")
                        nc.tensor.matmul(out=y1_ps[:], lhsT=w1_s[:], rhs=gTb[:],
                                         start=True, stop=True)
                        y1s = wpool.tile([P, P], F16, tag="y1s")
                        nc.scalar.activation(y1s[:], y1_ps[:], AF.Relu,
                                             bias=b1c_s[:])
                        y2_ps = psE.tile([P, P], F32, tag="epi")
                        nc.tensor.matmul(out=y2_ps[:], lhsT=w2_s[:], rhs=y1s[:],
                                         start=True, stop=True)
                        xo = wpool.tile([P, P], F32, tag="xo")
                        nc.vector.tensor_scalar(xo[:], y2_ps[:], b2c_s[:],
                                                None, OP.add)
                        nc.sync.dma_start(out=xoutT[:, w * P : (w + 1) * P],
                                          in_=xo[:])
                    return epi

                pending_epilogue[0] = make_epilogue(w, KT, numFM, exs_sb, mask_sb)

            emit_epilogue()

    nc.compile()
    return nc


def _preprocess(edge_index, edge_attr, ncores, nloc, nwin):
    """Degree-sorted identity-slot layout.

    Each core owns nloc dst nodes (snake-dealt by degree so cores balance).
    Within a core, dsts are sorted by degree desc; window w takes dsts
    [w*128, (w+1)*128), column r = rank within window. Edge j of the dst in
    column r sits at slot offs[w] + j*128 + r, so slot position == dst
    position in every 128-slot tile (scatter matrices become identity).
    KT profile is the per-window max tile count across cores (SPMD).
    """
    src = np.ascontiguousarray(edge_index[0]).astype(np.int64)
    dst = np.ascontiguousarray(edge_index[1]).astype(np.int64)
    n = nloc * ncores
    ea = np.ascontiguousarray(edge_attr, dtype=np.float32)

    deg = np.bincount(dst, minlength=n).astype(np.int64)
    deg2 = deg + 1  # + self loop

    # snake-deal ownership by degree
    dorder = np.argsort(-deg2, kind="stable")
    ranks = np.arange(n)
    blk = ranks // ncores
    cseq = np.where(blk % 2 == 0, ranks % ncores,
                    ncores - 1 - (ranks % ncores))
    owner = np.empty(n, np.int64)
    owner[dorder] = cseq

    # self-loop attr = mean of incoming edge attrs
    order = np.argsort(dst, kind="stable")
    dst_s = dst[order]
    src_s = src[order]
    ea_s = ea[order]
    cs = np.concatenate(
        [np.zeros((1, ea.shape[1]), np.float64),
         np.cumsum(ea_s, 0, dtype=np.float64)]
    )
    starts = np.searchsorted(dst_s, np.arange(n))
    ends = np.searchsorted(dst_s, np.arange(n) + 1)
    loop_attr = ((cs[ends] - cs[starts])
                 / np.maximum(deg, 1).astype(np.float64)[:, None]).astype(
        np.float32
    )

    # per-core window layout
    percore = []
    profs = np.zeros((ncores, nwin), np.int64)
    for c in range(ncores):
        own = np.where(owner == c)[0]
        o = np.argsort(-deg2[own], kind="stable")
        perm = own[o]  # global node id per column (window-major)
        d2 = deg2[perm]
        for w in range(nwin):
            seg = d2[w * P : (w + 1) * P]
            profs[c, w] = seg[0] if len(seg) else 1
        percore.append((perm, d2))

    ktprof = profs.max(0)
    ktprof = np.maximum(ktprof, 1)
    ktoff = np.concatenate([[0], np.cumsum(ktprof)])
    eslot = int(ktoff[-1]) * P

    data = []
    for c in range(ncores):
        perm, d2 = percore[c]
        ncol = len(perm)
        src_slot = np.zeros(eslot, np.int64)
        valid = np.zeros(eslot, bool)
        ea_slot = np.zeros((eslot, CDIM), np.float32)

        col_w = np.arange(ncol) // P
        col_r = np.arange(ncol) % P
        # edges grouped by dst: dst_s[starts[g]:ends[g]] are node g's edges
        for i in range(ncol):
            g = perm[i]
            w, r = col_w[i], col_r[i]
            base = int(ktoff[w]) * P + r
            k = int(deg[g])
            sl = base + np.arange(k) * P
            src_slot[sl] = src_s[starts[g] : ends[g]]
            ea_slot[sl] = ea_s[starts[g] : ends[g]]
            # self loop at edge index k
            sl2 = base + k * P
            src_slot[sl2] = g
            ea_slot[sl2] = loop_attr[g]
            valid[sl] = True
            valid[sl2] = True
        m = np.zeros((P, int(ktprof.sum()), 4), np.float16)
        for i in range(ncol):
            w, r = col_w[i], col_r[i]
            m[r, int(ktoff[w]) : int(ktoff[w]) + int(d2[i]), :] = 1.0
        mask01 = m.reshape(P, -1)

        eaT = np.ascontiguousarray(ea_slot.T).astype(NPF16)
        data.append(dict(src_slot=src_slot, valid=valid, eaT=eaT,
                         mask01=mask01, perm=perm))
    return data, [int(k) for k in ktprof]


def _layer_weight_maps(inputs, layer):
    i = layer
    att = np.asarray(inputs["att"])
    attf = att[i].reshape(-1).astype(np.float32)
    sgn = np.zeros((P, H), np.float32)
    for h in range(H):
        sgn[h * CDIM : (h + 1) * CDIM, h] = np.sign(
            attf[h * CDIM : (h + 1) * CDIM]
        )
    idt = np.zeros((P, 4 * P), NPFP8)
    eye = np.eye(P, dtype=NPFP8)
    for k in range(4):
        idt[:, k * P : (k + 1) * P] = eye
    hm = np.zeros((4, P), NPFP8)
    for h in range(H):
        hm[h, h * CDIM : (h + 1) * CDIM] = 1.0
    bgc = (np.asarray(inputs["bias"][i]) + np.asarray(inputs["bl"][i])).astype(
        np.float32
    )
    w1f = np.asarray(inputs["w1"][i]).astype(np.float32)
    b1p = np.asarray(inputs["b1"][i]).astype(np.float32) + bgc @ w1f
    m = dict(
        Wl_b=np.ascontiguousarray(inputs["Wl"][i]).astype(NPF16),
        Wr=np.ascontiguousarray(inputs["Wr"][i]).astype(NPF16),
        We_b=np.ascontiguousarray(inputs["We"][i]).astype(NPF16),
        w1=w1f.astype(NPF16),
        w2=np.ascontiguousarray(inputs["w2"][i]).astype(NPF16),
        attcol=np.abs(attf).reshape(P, 1).astype(NPF32),
        pbias=(np.abs(attf)
               * (np.asarray(inputs["br"][i]) + np.asarray(inputs["bl"][i])))
        .reshape(P, 1)
        .astype(NPF32),
        sgn4=sgn.astype(NPF16),
        ident=idt,
        hmask=hm,
        b1c=b1p.reshape(P, 1).astype(NPF32),
        b2c=np.asarray(inputs["b2"][i]).reshape(P, 1).astype(NPF32),
    )
    return m


_NC_CACHE = {}


def kernel(**inputs):
    nodes = np.asarray(inputs["nodes"], dtype=np.float32)
    edge_index = np.asarray(inputs["edge_index"])
    edge_attr = np.asarray(inputs["edge_attr"], dtype=np.float32)

    n, d = nodes.shape
    assert (n, d) == (N, D)
    nloc = n // NCORES
    nwin = -(-nloc // P)

    data, ktprof = _preprocess(edge_index, edge_attr, NCORES, nloc, nwin)

    key = (tuple(ktprof), NCORES)
    if key not in _NC_CACHE:
        _NC_CACHE[key] = build_layer_nc(dict(ktprof=ktprof, ncores=NCORES))
    nc = _NC_CACHE[key]

    x_curr = np.ascontiguousarray(nodes.T)  # [D, N] f32

    nlocp = nwin * P
    for layer in range(L):
        wmap = _layer_weight_maps(inputs, layer)
        xce = x_curr.astype(NPF16)
        in_maps = []
        for c in range(NCORES):
            dd = data[c]
            perm = dd["perm"]
            xTloc = np.zeros((P, nlocp), NPF16)
            xTloc[:, : len(perm)] = xce[:, perm]
            xgT = xce[:, dd["src_slot"]]
            xgT[:, ~dd["valid"]] = 0
            m = dict(wmap)
            m["xgT"] = np.ascontiguousarray(xgT)
            m["xTloc"] = xTloc
            m["eaT"] = dd["eaT"]
            m["maskd"] = dd["mask01"]
            in_maps.append(m)
        res = run_bass_kernel_spmd(
            nc, in_maps, core_ids=list(range(NCORES)), trace=TRACE
        )
        if res.exec_time_ns is not None:
            LAST_EXEC_NS.append(res.exec_time_ns)
        if TRACE:
            LAST_RESULTS.append(res)
        outs = res.results
        x_next = np.zeros((P, n), NPF32)
        for c in range(NCORES):
            perm = data[c]["perm"]
            x_next[:, perm] = outs[c]["xoutT"][:, : len(perm)]
        x_curr = x_next

    return np.ascontiguousarray(x_curr.T.astype(np.float32))
